# revision 6
# baseline (speedup 1.0000x reference)
"""Trainium2 Bass kernel for nn_FCNNShapeCounterValuationFunction.

Computes out[i] = 0.999 * a[i, int(z[i, 5])] for z:[B,32] f32, a:[B,16] f32.

Strategy (pure data parallel, 8 NeuronCores):
  - Shard rows across 8 cores (BC = B/8 rows each); per core, view rows as
    [128 partitions, BC/128] with per-partition contiguous blocks so every
    DMA descriptor is a large contiguous chunk.
  - Per round of F rows/partition: z rides the SP HWDGE ring, a the ACT
    ring; ACT extracts the index column; DVE does the 16-way gather as 16
    scalar_tensor_tensor ops prod[:,k,:] = (idx == k) * a[:,:,k], a binary
    tree add, and the 0.999 scale; GpSimd (SWDGE) issues output stores.
  - PACING: the chip's HBM is saturated by 8 cores streaming at once and
    the arbiter is unfair -- unpaced, some cores sustain ~430 GB/s and
    finish early while others straggle at ~300 GB/s (max-core 343 us vs
    fastest 275 us). Each round's z/a tiles are released for reuse only
    after small DVE "pacer" ops at the end of the round's compute, sized so
    the whole DVE round takes PACE_NS_ROW per row -- the fair-share DMA
    round time (bytes / (chip_HBM/8)). With loads gated on buffer reuse,
    per-core DMA issue is clocked at fair share, no core hogs, and all
    cores finish together near the aggregate HBM roofline. bufs=3 gives a
    two-round prefetch cushion against arbitration jitter.
"""

import numpy as np

B = 4194304
D = 32
K = 16
ATTR = 5
SCALE = 0.999
N_CORES = 8
P = 128
BC = B // N_CORES  # 524288 rows per core
F = 256  # rows per partition per round

# DVE cost model (HW-calibrated): ~1 elem/cycle/partition @0.96 GHz plus
# ~310 cycles dispatch overhead per instruction.
DVE_GHZ = 0.96
OVH = 275
# Fair-share pace: 196B/row (z 128 + a 64 + out 4) / ~358 GB/s/core -> 67
# ns/row of DVE clock per round.
PACE_NS_ROW = 69.0


def _round_sizes(npp, f=F):
    # Two half-size head rounds shorten the DMA ramp before the pace clock
    # starts ticking.
    head = [f // 2, f // 2]
    mid = npp - sum(head)
    assert mid >= 0 and mid % f == 0
    return head + [f] * (mid // f)


def _base_cycles(fr):
    # 16 STT + 3 tree + 1 final add + 1 scale = 21 ops, 32*fr elements.
    return 32 * fr + 21 * OVH


_cache = {}


def _build(bc=BC, f=F, pace_ns_row=PACE_NS_ROW, bufs=3):
    """Build + compile the per-core Bass program for bc rows."""
    from contextlib import ExitStack

    import concourse.tile as tile
    from concourse import bacc, mybir

    npp = bc // P  # rows per partition
    assert bc % P == 0
    rounds = _round_sizes(npp, f)
    nr = len(rounds)

    nc = bacc.Bacc("TRN2", target_bir_lowering=False, debug=False, num_devices=N_CORES)
    z = nc.dram_tensor("z", [bc, D], mybir.dt.float32, kind="ExternalInput")
    a = nc.dram_tensor("a", [bc, K], mybir.dt.float32, kind="ExternalInput")
    out = nc.dram_tensor("out", [bc], mybir.dt.float32, kind="ExternalOutput")

    # Partition-major views: partition p owns rows [p*npp, (p+1)*npp) so each
    # partition's DMA chunk is contiguous in DRAM.
    zv = z.ap().rearrange("(p n) d -> p n d", p=P)
    av = a.ap().rearrange("(p n) k -> p n k", p=P)
    ov = out.ap().rearrange("(p n) -> p n", p=P)

    f32 = mybir.dt.float32
    eq = mybir.AluOpType.is_equal
    mult = mybir.AluOpType.mult
    add = mybir.AluOpType.add

    with ExitStack() as ctx:
        tc = ctx.enter_context(tile.TileContext(nc))
        zpool = ctx.enter_context(tc.tile_pool(name="zpool", bufs=bufs))
        apool = ctx.enter_context(tc.tile_pool(name="apool", bufs=bufs))
        ppool = ctx.enter_context(tc.tile_pool(name="ppool", bufs=1))
        ipool = ctx.enter_context(tc.tile_pool(name="ipool", bufs=2))
        opool = ctx.enter_context(tc.tile_pool(name="opool", bufs=4))
        fpool = ctx.enter_context(tc.tile_pool(name="fpool", bufs=1))

        scr = fpool.tile([P, 2048], f32, tag="scr", name="scr")

        pos = 0
        for r, fr in enumerate(rounds):
            lo, hi = pos, pos + fr
            pos = hi

            # a rides the ACT HWDGE ring, z the SP ring: two descriptor
            # generators feed the 16 SDMA engines in parallel.
            at = apool.tile([P, fr, K], f32, tag="at", name="at")
            nc.scalar.dma_start(at[:], av[:, lo:hi, :])
            zt = zpool.tile([P, fr, D], f32, tag="zt", name="zt")
            nc.sync.dma_start(zt[:], zv[:, lo:hi, :])

            # idx collects the index column (ACT engine, strided read).
            idx = ipool.tile([P, fr], f32, tag="idx", name="idx")
            nc.scalar.copy(idx[:], zt[:, :, ATTR])

            # prod[:, k, :] = (idx == k) * a[:, :, k]  (k-major: contiguous)
            prod = ppool.tile([P, K, fr], f32, tag="prod", name="prod")
            for k in range(K):
                nc.vector.scalar_tensor_tensor(
                    prod[:, k, :], idx[:], float(k), at[:, :, k], eq, mult
                )

            # In-place binary-tree sum over k: all operands contiguous.
            for h in (8, 4, 2):
                nc.vector.tensor_tensor(
                    prod[:, :h, :], prod[:, :h, :], prod[:, h : 2 * h, :], add
                )
            red = ipool.tile([P, fr], f32, tag="red", name="red")
            nc.vector.tensor_tensor(red[:], prod[:, 0, :], prod[:, 1, :], add)

            # Scale on DVE; store via GpSimd SWDGE so the load rings never
            # wait on compute. opool bufs=4 keeps SWDGE store latency off
            # the DVE clock path.
            sc = opool.tile([P, fr], f32, tag="sc", name="sc")
            nc.vector.tensor_scalar_mul(sc[:], red[:], SCALE)
            nc.gpsimd.dma_start(ov[:, lo:hi], sc[:])

            # Pacer ops: tail-slice re-reads of this round's at and zt on
            # DVE, sized so the full DVE round takes pace_ns_row per row.
            # Their completion releases the tiles for round r+bufs's loads,
            # clocking per-core DMA issue at fair share. Only emitted where
            # a gated load exists (r + bufs < nr).
            if r + bufs < nr:
                budget = int(fr * pace_ns_row * DVE_GHZ) - _base_cycles(fr)
                budget -= 2 * OVH
                if budget > 64:
                    ma = max(1, min(fr, budget // 2 // K))  # rows of at
                    mz = max(1, min(fr, budget // 2 // D))  # rows of zt
                    nc.vector.tensor_scalar_mul(
                        scr[:, : ma * K].rearrange("p (f k) -> p f k", k=K),
                        at[:, fr - ma :, :],
                        1.0,
                    )
                    nc.vector.tensor_scalar_mul(
                        scr[:, : mz * D].rearrange("p (f d) -> p f d", d=D),
                        zt[:, fr - mz :, :],
                        1.0,
                    )
                else:
                    # Tiny budget: single minimal touches of both tiles.
                    nc.vector.tensor_scalar_mul(
                        scr[:, :K].rearrange("p (f k) -> p f k", k=K),
                        at[:, fr - 1 :, :],
                        1.0,
                    )
                    nc.vector.tensor_scalar_mul(
                        scr[:, :D].rearrange("p (f d) -> p f d", d=D),
                        zt[:, fr - 1 :, :],
                        1.0,
                    )

    nc.compile()
    return nc


def _get(bc=BC, f=F, pace_ns_row=PACE_NS_ROW, bufs=3):
    key = (bc, f, pace_ns_row, bufs)
    if key not in _cache:
        _cache[key] = _build(bc, f, pace_ns_row=pace_ns_row, bufs=bufs)
    return _cache[key]


def kernel(z, a, attr_index=5, **run_kwargs):
    """Full inputs in, full output out. Shards rows over 8 NeuronCores."""
    from concourse import bass_utils

    assert int(attr_index) == ATTR
    z = np.asarray(z, dtype=np.float32)
    a = np.asarray(a, dtype=np.float32)
    assert z.shape == (B, D) and a.shape == (B, K)

    nc = _get()
    in_maps = [
        {"z": z[c * BC : (c + 1) * BC], "a": a[c * BC : (c + 1) * BC]}
        for c in range(N_CORES)
    ]
    res = bass_utils.run_bass_kernel_spmd(
        nc, in_maps, core_ids=list(range(N_CORES)), **run_kwargs
    )
    out = np.concatenate([r["out"] for r in res.results], axis=0)
    if run_kwargs:
        kernel.last_results = res
    return out
